# revision 3
# baseline (speedup 1.0000x reference)
"""Neural CDE RK4 on 8 TRN2 cores — V5: two interleaved batch streams per core.

Measured (probe3): single-stream W=128 chain latency is ~3.35us/substep (the
V3 baseline sits at this floor); two independent W=64 streams with fully
separate PSUM banks advance one global substep every ~2.7us. V5 restructures
the kernel as 2 streams of 64 batch columns each:

- per stream, one PSUM bank holds the four z1-preactivation slots at
  partition offsets 0/32/64/96 (32-aligned reads), so ONE base matmul
  [33,112] per step writes all four W1^T.h bases, and the per-substep
  reduction is a single "main pair" (lhsT [96,<=112], zero-padded columns)
  that adds the on-path c_s*W1^T.k into slot s+1 AND the RK4-combine
  (w_s*W1^T.k) into slot 0 for the next step.
- h lives in a shared PSUM bank (stream k at partitions 32k), accumulated by
  a per-substep "ph pair" (lhsT = w_s * tile(I,3)); never reset.
- dx broadcast tiles are precomputed on the host and streamed by DMA
  (no sel matmuls / scalar-engine copies on device).
"""

import numpy as np

import concourse.bass as bass
import concourse.tile as tile
from concourse import bacc, mybir
from concourse.bass_utils import run_bass_kernel_spmd

B, T, D, H, HH = 1024, 1000, 6, 32, 15
NCORES = 8
P = B // NCORES          # 128 batch per core
NS = 2                   # streams per core
W = P // NS              # 64 batch per stream
TS_FULL = T - 1          # 999 scan steps
CH = 24                  # steps per bc DMA chunk

F32 = mybir.dt.float32
F16 = mybir.dt.float16
TANH = mybir.ActivationFunctionType.Tanh

MW = [48, 80, 112, 16]   # main-pair output widths per substep
WGT = [1.0 / 6.0, 1.0 / 3.0, 1.0 / 3.0, 1.0 / 6.0]
CSUB = [0.5, 0.5, 1.0]


def _emit(tc, ins, out_ap, ts):
    nc = tc.nc
    nchunk = (ts + CH - 1) // CH
    with tc.tile_pool(name="sb", bufs=1) as sb, \
         tc.tile_pool(name="ps", bufs=1, space="PSUM") as ps:

        # ---- PSUM (7 banks) ----
        slots = [ps.tile([128, W], F32, name=f"slots{k}") for k in range(NS)]
        phb = ps.tile([64, 2 * W], F32, name="phb")   # stream k rows 32k; half-sums in col blocks
        z23 = [ps.tile([15, 2 * W], F32, name=f"z23_{k}") for k in range(NS)]
        pf = [ps.tile([96, 2 * W], F32, name=f"pf{k}") for k in range(NS)]

        # ---- SBUF ----
        w2t = sb.tile([96, HH], F16, name="w2t")
        w3t = sb.tile([96, HH], F16, name="w3t")
        w4a = sb.tile([96, 96], F16, name="w4a")
        w4b = sb.tile([96, 96], F16, name="w4b")
        w1q = sb.tile([96, 112], F16, name="w1q")
        mr = [sb.tile([96, MW[s]], F16, name=f"mr{s}") for s in range(4)]
        phw = [sb.tile([96, 32], F16, name=f"phw{s}") for s in range(4)]
        eye32 = sb.tile([32, 32], F32, name="eye32")
        h0f = [sb.tile([32, 2 * W], F32, name=f"h0f{k}") for k in range(NS)]
        h16 = [sb.tile([96, W], F16, name=f"h16_{k}") for k in range(NS)]
        z1 = [sb.tile([96, W], F16, name=f"z1_{k}") for k in range(NS)]
        z2 = [sb.tile([96, W], F16, name=f"z2_{k}") for k in range(NS)]
        z3 = [sb.tile([96, W], F16, name=f"z3_{k}") for k in range(NS)]
        tt = [sb.tile([96, 2 * W], F16, name=f"tt{k}") for k in range(NS)]
        uu = [sb.tile([96, 2 * W], F16, name=f"uu{k}") for k in range(NS)]
        bcch = [sb.tile([96, CH * 2 * P], F16, name=f"bcch{i}") for i in range(2)]
        hout = sb.tile([32, P], F32, name="hout")

        # ---- one-time loads ----
        for t_sb, name in [(w2t, "w2t"), (w3t, "w3t"), (w4a, "w4a"),
                           (w4b, "w4b"), (w1q, "w1q"), (eye32, "eye32")]:
            nc.sync.dma_start(out=t_sb[:, :], in_=ins[name][:, :])
        for s in range(4):
            nc.sync.dma_start(out=mr[s][:, :], in_=ins[f"mr{s}"][:, :])
            nc.sync.dma_start(out=phw[s][:, :], in_=ins[f"phw{s}"][:, :])
        for k in range(NS):
            nc.sync.dma_start(out=h0f[k][:, :], in_=ins[f"h0f{k}"][:, :])
            nc.vector.memset(h16[k][:, :], 0.0)
            nc.sync.dma_start(out=h16[k][0:33, :], in_=ins[f"h0t{k}"][:, :])
            for zt in (z1[k], z2[k], z3[k]):
                nc.vector.memset(zt[:, :], 0.0)
                nc.sync.dma_start(out=zt[15:16, :], in_=ins["ones16"][:, :])
        nc.sync.dma_start(out=bcch[0][:, :], in_=ins["bcc"][0, :, :])
        if nchunk > 1:
            nc.sync.dma_start(out=bcch[1][:, :], in_=ins["bcc"][1, :, :])

        # ---- seeds ----
        for k in range(NS):
            nc.tensor.matmul(phb[32 * k:32 * k + 32, :], lhsT=eye32[:, :],
                             rhs=h0f[k][:, :], start=True, stop=False,
                             skip_group_check=True)
            nc.tensor.matmul(slots[k][0:112, :], lhsT=w1q[:, :],
                             rhs=h16[k][:, :], start=True, stop=False,
                             skip_group_check=True)

        for t in range(ts):
            ci = t // CH + 1
            if t % CH == 0 and 2 <= ci < nchunk:
                nc.sync.dma_start(out=bcch[ci % 2][:, :], in_=ins["bcc"][ci, :, :])
            cb = bcch[(t // CH) % 2]
            bo = (t % CH) * 2 * P
            for s in range(4):
                for k in range(NS):
                    nc.vector.tensor_scalar_max(
                        z1[k][0:15, :], slots[k][32 * s:32 * s + 15, :], 0.0)
                for k in range(NS):
                    nc.tensor.matmul(z23[k][:, 0:W], lhsT=w2t[:, :],
                                     rhs=z1[k][:, :], start=True, stop=True,
                                     skip_group_check=True)
                for k in range(NS):
                    nc.vector.tensor_scalar_max(z2[k][0:15, :], z23[k][:, 0:W], 0.0)
                if s == 0:
                    for k in range(NS):
                        nc.vector.tensor_copy(hout[:, W * k:W * k + W],
                                              phb[32 * k:32 * k + 32, 0:W])
                    for k in range(NS):
                        nc.vector.tensor_add(h16[k][0:32, :],
                                             hout[:, W * k:W * k + W],
                                             phb[32 * k:32 * k + 32, W:2 * W])
                for k in range(NS):
                    nc.tensor.matmul(z23[k][:, W:2 * W], lhsT=w3t[:, :],
                                     rhs=z2[k][:, :], start=True, stop=True,
                                     skip_group_check=True)
                if s == 0:
                    for k in range(NS):
                        nc.tensor.matmul(slots[k][0:112, :], lhsT=w1q[:, :],
                                         rhs=h16[k][:, :], start=True,
                                         stop=False, skip_group_check=True)
                for k in range(NS):
                    nc.vector.tensor_scalar_max(z3[k][0:15, :],
                                                z23[k][:, W:2 * W], 0.0)
                for k in range(NS):
                    nc.tensor.matmul(pf[k][:, 0:W], lhsT=w4a[:, :],
                                     rhs=z3[k][:, :], start=True, stop=True,
                                     skip_group_check=True)
                for k in range(NS):
                    nc.tensor.matmul(pf[k][:, W:2 * W], lhsT=w4b[:, :],
                                     rhs=z3[k][:, :], start=True, stop=True,
                                     skip_group_check=True)
                for k in range(NS):
                    nc.scalar.activation(tt[k][:, :], pf[k][:, :], TANH)
                for k in range(NS):
                    nc.vector.tensor_mul(
                        uu[k][:, :], tt[k][:, :],
                        cb[:, bo + 2 * W * k:bo + 2 * W * k + 2 * W])
                for k in range(NS):
                    mw = MW[s]
                    nc.tensor.matmul(slots[k][0:mw, :], lhsT=mr[s][:, :],
                                     rhs=uu[k][:, 0:W], start=False, stop=False,
                                     skip_group_check=True)
                    nc.tensor.matmul(slots[k][0:mw, :], lhsT=mr[s][:, :],
                                     rhs=uu[k][:, W:2 * W], start=False,
                                     stop=False, skip_group_check=True)
                for k in range(NS):
                    nc.tensor.matmul(phb[32 * k:32 * k + 32, :], lhsT=phw[s][:, :],
                                     rhs=uu[k][:, :], start=False, stop=False,
                                     skip_group_check=True)

        for k in range(NS):
            nc.vector.tensor_copy(hout[:, W * k:W * k + W],
                                  phb[32 * k:32 * k + 32, 0:W])
            nc.vector.tensor_add(hout[:, W * k:W * k + W],
                                 hout[:, W * k:W * k + W],
                                 phb[32 * k:32 * k + 32, W:2 * W])
        nc.sync.dma_start(out=out_ap[:, :], in_=hout[:, :])


_CACHE = {}


def _input_specs(ts):
    nchunk = (ts + CH - 1) // CH
    specs = {
        "w2t": ((96, HH), F16), "w3t": ((96, HH), F16),
        "w4a": ((96, 96), F16), "w4b": ((96, 96), F16),
        "w1q": ((96, 112), F16), "eye32": ((32, 32), F32),
        "ones16": ((1, W), F16),
        "bcc": ((nchunk, 96, CH * 2 * P), F16),
    }
    for s in range(4):
        specs[f"mr{s}"] = ((96, MW[s]), F16)
        specs[f"phw{s}"] = ((96, 32), F16)
    for k in range(NS):
        specs[f"h0f{k}"] = ((32, 2 * W), F32)
        specs[f"h0t{k}"] = ((33, W), F16)
    return specs


def build(ts=TS_FULL):
    if ts in _CACHE:
        return _CACHE[ts]
    nc = bacc.Bacc("TRN2", target_bir_lowering=False, debug=False,
                   enable_asserts=False, num_devices=NCORES)
    ins = {
        name: nc.dram_tensor(name, list(shape), dt, kind="ExternalInput").ap()
        for name, (shape, dt) in _input_specs(ts).items()
    }
    out_ap = nc.dram_tensor("ht_out", [H, P], F32, kind="ExternalOutput").ap()
    with tile.TileContext(nc, trace_sim=False) as tc:
        _emit(tc, ins, out_ap, ts)
    nc.compile()
    _CACHE[ts] = nc
    return nc


def host_prep(coeffs, W0, b0, W1, b1, W2, b2, W3, b3, W4, b4, ts=TS_FULL):
    f32, f16 = np.float32, np.float16
    coeffs = np.ascontiguousarray(coeffs, dtype=f32)
    h0 = coeffs[:, 0, :] @ W0.astype(f32) + b0.astype(f32)      # [B, H]
    dX = coeffs[:, 1:ts + 1, :] - coeffs[:, :ts, :]             # [B, ts, D]

    W1 = W1.astype(f32)
    W4r = W4.astype(f32).reshape(HH, H, D)
    W4P = W4r.transpose(0, 2, 1).reshape(HH, D * H)             # cols d*32+i
    b4P = b4.astype(f32).reshape(H, D).T.reshape(D * H)
    RW1 = np.tile(W1, (3, 1)).astype(f32)                       # [96, 15]
    Rsel = np.tile(np.eye(H, dtype=f32), (3, 1))                # [96, 32]

    w1q = np.zeros((96, 112), f32)
    for c in range(4):
        w1q[0:32, 32 * c:32 * c + 15] = W1
        w1q[32, 32 * c:32 * c + 15] = b1.astype(f32)

    def pad96(m):
        out = np.zeros((96, m.shape[1]), f32)
        out[:m.shape[0]] = m
        return out

    shared = {
        "w2t": pad96(np.concatenate([W2.astype(f32), b2.astype(f32)[None]], 0)),
        "w3t": pad96(np.concatenate([W3.astype(f32), b3.astype(f32)[None]], 0)),
        "w4a": pad96(np.concatenate([W4P[:, :96], b4P[None, :96]], 0)),
        "w4b": pad96(np.concatenate([W4P[:, 96:], b4P[None, 96:]], 0)),
        "w1q": w1q,
        "ones16": np.ones((1, W), f32),
    }
    for s in range(4):
        m = np.zeros((96, MW[s]), f32)
        m[:, 0:15] = WGT[s] * RW1
        if s < 3:
            m[:, 32 * (s + 1):32 * (s + 1) + 15] = CSUB[s] * RW1
        shared[f"mr{s}"] = m
        shared[f"phw{s}"] = WGT[s] * Rsel
    shared = {k: np.ascontiguousarray(v, f16) for k, v in shared.items()}
    shared["eye32"] = np.eye(32, dtype=f32)

    nchunk = (ts + CH - 1) // CH
    in_maps = []
    for c in range(NCORES):
        sl = slice(c * P, (c + 1) * P)
        h0c = h0[sl]                                             # [P, H]
        dxc = dX[sl]                                             # [P, ts, D]
        m = dict(shared)
        bc_all = np.zeros((nchunk * CH, 96, 2 * P), f32)
        for k in range(NS):
            h0k = np.ascontiguousarray(h0c[W * k:W * k + W].T, f32)  # [32, W]
            h0kp = np.zeros((32, 2 * W), f32)
            h0kp[:, 0:W] = h0k
            m[f"h0f{k}"] = h0kp
            m[f"h0t{k}"] = np.ascontiguousarray(
                np.concatenate([h0k, np.ones((1, W), f32)], 0), f16)
            arr = dxc[W * k:W * k + W].transpose(1, 2, 0)        # [ts, D, W]
            for hf in range(2):
                sub = arr[:, 3 * hf:3 * hf + 3, :]               # [ts, 3, W]
                rep = np.repeat(sub[:, :, None, :], 32, axis=2)  # [ts,3,32,W]
                bc_all[:ts, :, 2 * W * k + W * hf:2 * W * k + W * hf + W] = \
                    rep.reshape(ts, 96, W)
        bcc = bc_all.reshape(nchunk, CH, 96, 2 * P).transpose(0, 2, 1, 3)
        m["bcc"] = np.ascontiguousarray(
            bcc.reshape(nchunk, 96, CH * 2 * P), f16)
        in_maps.append(m)
    return in_maps


def run_device(in_maps, ts=TS_FULL, **kw):
    nc = build(ts)
    return run_bass_kernel_spmd(nc, in_maps, list(range(NCORES)), **kw)


def kernel(coeffs, W0, b0, W1, b1, W2, b2, W3, b3, W4, b4, Wf, bf):
    in_maps = host_prep(coeffs, W0, b0, W1, b1, W2, b2, W3, b3, W4, b4)
    res = run_device(in_maps)
    hT = np.stack([res.results[c]["ht_out"] for c in range(NCORES)])  # [8,H,P]
    h_all = hT.transpose(0, 2, 1).reshape(B, H)
    return (h_all @ Wf.astype(np.float32) + bf.astype(np.float32)).astype(
        np.float32)
